# revision 32
# baseline (speedup 1.0000x reference)
"""AttentivePooling Trainium2 kernel.

Reference semantics (h_all: [T, B, D] f32, xin unused):
    h_last = h_all[-1]                       # [B, D]
    a[b, t] = <h_all[t, b, :], h_last[b, :]> / sqrt(D)
    r = relu(a)
    w = r / (sum_t r + 1e-9)
    out[b, d] = sum_t w[b, t] * h_all[t, b, d]

Strategy: data-parallel over B across 8 cores (8 batches/core, no
collectives).  Per batch on-device:
  - one 4MB SWDGE DMA loads h_b as 16 SBUF chunks [128(t), 512(d)]
    (t = c*128 + p); a single HWDGE dma_start with 2048 descriptors
    wedges the exec unit, SWDGE handles it
  - PE broadcasts h_last across 128 partitions into PSUM
    (ones[1,128].T @ hl[1,512]); the DVE multiply reads it from PSUM
  - scores: DVE tensor_tensor multiply; the free-dim reduction is
    split between ACT (activation accum_out, scale=1/sqrt(D) folded)
    and DVE (tensor_reduce + tensor_scalar rescale) to balance load.
    (The fused DVE tensor_tensor_reduce crashes the exec unit on HW.)
  - ACT relu with accum_out produces weights + their per-partition sums
  - PE accumulates sum_t w_t * h_t into PSUM [1, 512] (w stationary)
  - PE reduces the weight-sum across partitions via a ones column
  - DVE computes 1/(Z + 1e-9); ACT scales the pooled vector
"""

import numpy as np
from contextlib import ExitStack

import concourse.bass as bass
import concourse.tile as tile
from concourse import bacc, mybir
from concourse.bass_utils import run_bass_kernel_spmd

T, B, D = 2048, 64, 512
NCORES = 8
BPC = B // NCORES  # batches per core
P = 128
TC = T // P  # 16 T-chunks per batch
SCALE = float(1.0 / np.sqrt(np.float32(D)))
DVE_REDUCE_CHUNKS = frozenset({3, 6, 9, 12, 15})  # reduces on DVE; rest on ACT
GP_MULT_CHUNKS = frozenset({2, 5, 8, 11, 14})  # multiplies on GPSIMD; rest on DVE

_nc_cache = None


def _build():
    global _nc_cache
    if _nc_cache is not None:
        return _nc_cache
    nc = bacc.Bacc("TRN2", debug=False, target_bir_lowering=False, num_devices=NCORES)
    h = nc.dram_tensor("h", [T, BPC, D], mybir.dt.float32r, kind="ExternalInput")
    out = nc.dram_tensor("out", [BPC, D], mybir.dt.float32, kind="ExternalOutput")
    h_ap = h.ap()
    out_ap = out.ap()
    f32 = mybir.dt.float32
    f32r = mybir.dt.float32r

    with tile.TileContext(nc) as tc:
        with ExitStack() as ctx:
            hpool = ctx.enter_context(tc.tile_pool(name="h", bufs=4))
            psbcp = ctx.enter_context(tc.tile_pool(name="psb", bufs=2, space="PSUM"))
            tmpp = ctx.enter_context(tc.tile_pool(name="tmp", bufs=4))
            smallp = ctx.enter_context(tc.tile_pool(name="small", bufs=3))
            constp = ctx.enter_context(tc.tile_pool(name="const", bufs=1))
            psoutp = ctx.enter_context(tc.tile_pool(name="pso", bufs=3, space="PSUM"))
            pszp = ctx.enter_context(tc.tile_pool(name="psz", bufs=3, space="PSUM"))

            ones_col = constp.tile([P, 1], f32)
            nc.vector.memset(ones_col[:], 1.0)
            eps_tile = constp.tile([1, 1], f32)
            nc.vector.memset(eps_tile[:], 1e-9)

            def bcast(b):
                # broadcast h_last[b] to all 128 partitions via
                # partition-stride-0 DMA reads straight from DRAM.
                # One copy lands in PSUM (DVE reads it via the PSUM port,
                # halving SBUF read-port pressure), one in SBUF (GPSIMD
                # cannot read PSUM).
                src_bc = h_ap[T - 1 : T, b, :].bitcast(f32).broadcast_to([P, D])
                hlb = smallp.tile([P, D], f32, tag="hlb", name="hlb")
                nc.sync.dma_start(hlb[:], src_bc)
                psb = psbcp.tile([P, D], f32, tag="psb")
                nc.scalar.copy(psb[:], hlb[:])
                return psb, hlb

            def load_h(b):
                # 16 per-chunk HWDGE DMAs: chunk-granular deps (scores can
                # start as soon as a chunk lands) and no GPSIMD SWDGE
                # descriptor-generation cost.
                t = hpool.tile([P, TC, D], f32r, tag="hsb", name="h_sb")
                for c in range(TC):
                    nc.sync.dma_start(t[:, c, :], h_ap[c * P : (c + 1) * P, b, :])
                return t

            h_tiles = {}
            for b in range(min(2, BPC)):
                h_tiles[b] = load_h(b)
            psb_tiles = {0: bcast(0)}

            for b in range(BPC):
                h_sb = h_tiles.pop(b)
                psb, hlb = psb_tiles.pop(b)

                # scores: scr[p, c] = sum_d h[t, d] * hl[d] * SCALE
                scr = smallp.tile([P, TC], f32, tag="scr")
                for c in range(TC):
                    tmp = tmpp.tile([P, D], f32, tag="tmp")
                    if c in GP_MULT_CHUNKS:
                        nc.gpsimd.tensor_tensor(
                            tmp[:],
                            h_sb[:, c, :].bitcast(f32),
                            hlb[:],
                            mybir.AluOpType.mult,
                        )
                    else:
                        nc.vector.tensor_tensor(
                            tmp[:],
                            h_sb[:, c, :].bitcast(f32),
                            psb[:],
                            mybir.AluOpType.mult,
                        )
                    if c in DVE_REDUCE_CHUNKS:
                        nc.vector.tensor_reduce(
                            scr[:, c : c + 1],
                            tmp[:],
                            mybir.AxisListType.X,
                            mybir.AluOpType.add,
                        )
                    else:
                        nc.scalar.activation(
                            tmp[:],
                            tmp[:],
                            mybir.ActivationFunctionType.Copy,
                            scale=SCALE,
                            accum_out=scr[:, c : c + 1],
                        )
                # rescale the DVE-reduced columns (ACT ones had SCALE folded)
                for c in sorted(DVE_REDUCE_CHUNKS):
                    nc.vector.tensor_scalar_mul(
                        scr[:, c : c + 1], scr[:, c : c + 1], SCALE
                    )

                # relu + per-partition sum of relu'd scores
                w = smallp.tile([P, TC], f32r, tag="w")
                zcol = smallp.tile([P, 1], f32, tag="z")
                nc.scalar.activation(
                    w[:], scr[:], mybir.ActivationFunctionType.Relu, accum_out=zcol[:]
                )

                # next batch's broadcast goes to PE BEFORE this batch's
                # pooling burst, so the next scores phase is not blocked
                # behind the pooling in PE program order
                if b + 1 < BPC:
                    psb_tiles[b + 1] = bcast(b + 1)
                if b + 2 < BPC:
                    h_tiles[b + 2] = load_h(b + 2)

                # pooled[d] = sum_t w_t * h[t, d] accumulated over chunks
                pout = psoutp.tile([1, D], f32)
                for c in range(TC):
                    nc.tensor.matmul(
                        pout[:],
                        w[:, c : c + 1],
                        h_sb[:, c, :],
                        start=(c == 0),
                        stop=(c == TC - 1),
                    )
                # Z = sum over all t of relu'd scores
                pz = pszp.tile([1, 1], f32)
                nc.tensor.matmul(pz[:], zcol[:], ones_col[:], start=True, stop=True)

                zeps = smallp.tile([1, 1], f32, tag="zeps")
                nc.scalar.activation(
                    zeps[:],
                    pz[:],
                    mybir.ActivationFunctionType.Identity,
                    bias=eps_tile[0:1, 0:1],
                )
                zrec = smallp.tile([1, 1], f32, tag="zrec")
                nc.vector.reciprocal(zrec[:], zeps[:])
                res = smallp.tile([1, D], f32, tag="res")
                nc.scalar.mul(res[:], pout[:], zrec[0:1, 0:1])
                nc.sync.dma_start(out_ap[b : b + 1, :], res[:])

    nc.finalize()
    _nc_cache = nc
    return nc


def _run(h_all: np.ndarray, trace: bool = False):
    nc = _build()
    h_all = np.ascontiguousarray(np.asarray(h_all), dtype=np.float32)
    assert h_all.shape == (T, B, D)
    in_maps = [
        {"h": np.ascontiguousarray(h_all[:, c * BPC : (c + 1) * BPC, :])}
        for c in range(NCORES)
    ]
    r = run_bass_kernel_spmd(nc, in_maps, list(range(NCORES)), trace=trace)
    out = np.concatenate([r.results[c]["out"] for c in range(NCORES)], axis=0)
    return out, r


def kernel(h_all: np.ndarray, xin: np.ndarray | None = None) -> np.ndarray:
    out, _ = _run(h_all)
    return out


# revision 33
# speedup vs baseline: 1.1280x; 1.1280x over previous
"""AttentivePooling Trainium2 kernel.

Reference semantics (h_all: [T, B, D] f32, xin unused):
    h_last = h_all[-1]                       # [B, D]
    a[b, t] = <h_all[t, b, :], h_last[b, :]> / sqrt(D)
    r = relu(a)
    w = r / (sum_t r + 1e-9)
    out[b, d] = sum_t w[b, t] * h_all[t, b, d]

Strategy: data-parallel over B across 8 cores (8 batches/core, no
collectives).  Per batch on-device:
  - one 4MB SWDGE DMA loads h_b as 16 SBUF chunks [128(t), 512(d)]
    (t = c*128 + p); a single HWDGE dma_start with 2048 descriptors
    wedges the exec unit, SWDGE handles it
  - PE broadcasts h_last across 128 partitions into PSUM
    (ones[1,128].T @ hl[1,512]); the DVE multiply reads it from PSUM
  - scores: DVE tensor_tensor multiply; the free-dim reduction is
    split between ACT (activation accum_out, scale=1/sqrt(D) folded)
    and DVE (tensor_reduce + tensor_scalar rescale) to balance load.
    (The fused DVE tensor_tensor_reduce crashes the exec unit on HW.)
  - ACT relu with accum_out produces weights + their per-partition sums
  - PE accumulates sum_t w_t * h_t into PSUM [1, 512] (w stationary)
  - PE reduces the weight-sum across partitions via a ones column
  - DVE computes 1/(Z + 1e-9); ACT scales the pooled vector
"""

import numpy as np
from contextlib import ExitStack

import concourse.bass as bass
import concourse.tile as tile
from concourse import bacc, mybir
from concourse.bass_utils import run_bass_kernel_spmd

T, B, D = 2048, 64, 512
NCORES = 8
BPC = B // NCORES  # batches per core
P = 128
TC = T // P  # 16 T-chunks per batch
SCALE = float(1.0 / np.sqrt(np.float32(D)))
DVE_REDUCE_CHUNKS = frozenset({3, 6, 9, 12, 15})  # reduces on DVE; rest on ACT
GP_MULT_CHUNKS = frozenset({2, 5, 8, 11, 14})  # multiplies on GPSIMD; rest on DVE

_nc_cache = None


def _build():
    global _nc_cache
    if _nc_cache is not None:
        return _nc_cache
    nc = bacc.Bacc("TRN2", debug=False, target_bir_lowering=False, num_devices=NCORES)
    h = nc.dram_tensor("h", [T, BPC, D], mybir.dt.float32r, kind="ExternalInput")
    out = nc.dram_tensor("out", [BPC, D], mybir.dt.float32, kind="ExternalOutput")
    h_ap = h.ap()
    out_ap = out.ap()
    f32 = mybir.dt.float32
    f32r = mybir.dt.float32r

    with tile.TileContext(nc) as tc:
        with ExitStack() as ctx:
            hpool = ctx.enter_context(tc.tile_pool(name="h", bufs=4))
            psbcp = ctx.enter_context(tc.tile_pool(name="psb", bufs=2, space="PSUM"))
            tmpp = ctx.enter_context(tc.tile_pool(name="tmp", bufs=4))
            smallp = ctx.enter_context(tc.tile_pool(name="small", bufs=3))
            constp = ctx.enter_context(tc.tile_pool(name="const", bufs=1))
            psoutp = ctx.enter_context(tc.tile_pool(name="pso", bufs=3, space="PSUM"))
            pszp = ctx.enter_context(tc.tile_pool(name="psz", bufs=3, space="PSUM"))

            ones_col = constp.tile([P, 1], f32)
            nc.vector.memset(ones_col[:], 1.0)
            eps_tile = constp.tile([1, 1], f32)
            nc.vector.memset(eps_tile[:], 1e-9)

            def bcast(b):
                # broadcast h_last[b] to all 128 partitions via
                # partition-stride-0 DMA reads straight from DRAM.
                # One copy lands in PSUM (DVE reads it via the PSUM port,
                # halving SBUF read-port pressure), one in SBUF (GPSIMD
                # cannot read PSUM).
                src_bc = h_ap[T - 1 : T, b, :].bitcast(f32).broadcast_to([P, D])
                hlb = smallp.tile([P, D], f32, tag="hlb", name="hlb")
                nc.sync.dma_start(hlb[:], src_bc)
                psb = psbcp.tile([P, D], f32, tag="psb")
                nc.scalar.copy(psb[:], hlb[:])
                return psb, hlb

            def load_h(b):
                t = hpool.tile([P, TC, D], f32r, tag="hsb", name="h_sb")
                nc.gpsimd.dma_start(
                    t[:], h_ap[:, b, :].rearrange("(c p) d -> p c d", p=P)
                )
                return t

            h_tiles = {}
            for b in range(min(2, BPC)):
                h_tiles[b] = load_h(b)
            psb_tiles = {0: bcast(0)}

            for b in range(BPC):
                h_sb = h_tiles.pop(b)
                psb, hlb = psb_tiles.pop(b)

                # scores: scr[p, c] = sum_d h[t, d] * hl[d] * SCALE
                scr = smallp.tile([P, TC], f32, tag="scr")
                for c in range(TC):
                    tmp = tmpp.tile([P, D], f32, tag="tmp")
                    if c in GP_MULT_CHUNKS:
                        nc.gpsimd.tensor_tensor(
                            tmp[:],
                            h_sb[:, c, :].bitcast(f32),
                            hlb[:],
                            mybir.AluOpType.mult,
                        )
                    else:
                        nc.vector.tensor_tensor(
                            tmp[:],
                            h_sb[:, c, :].bitcast(f32),
                            psb[:],
                            mybir.AluOpType.mult,
                        )
                    if c in DVE_REDUCE_CHUNKS:
                        nc.vector.tensor_reduce(
                            scr[:, c : c + 1],
                            tmp[:],
                            mybir.AxisListType.X,
                            mybir.AluOpType.add,
                        )
                    else:
                        nc.scalar.activation(
                            tmp[:],
                            tmp[:],
                            mybir.ActivationFunctionType.Copy,
                            scale=SCALE,
                            accum_out=scr[:, c : c + 1],
                        )
                # rescale the DVE-reduced columns (ACT ones had SCALE folded)
                for c in sorted(DVE_REDUCE_CHUNKS):
                    nc.vector.tensor_scalar_mul(
                        scr[:, c : c + 1], scr[:, c : c + 1], SCALE
                    )

                # relu + per-partition sum of relu'd scores
                w = smallp.tile([P, TC], f32r, tag="w")
                zcol = smallp.tile([P, 1], f32, tag="z")
                nc.scalar.activation(
                    w[:], scr[:], mybir.ActivationFunctionType.Relu, accum_out=zcol[:]
                )

                # next batch's broadcast goes to PE BEFORE this batch's
                # pooling burst, so the next scores phase is not blocked
                # behind the pooling in PE program order
                if b + 1 < BPC:
                    psb_tiles[b + 1] = bcast(b + 1)
                if b + 2 < BPC:
                    h_tiles[b + 2] = load_h(b + 2)

                # pooled[d] = sum_t w_t * h[t, d] accumulated over chunks
                pout = psoutp.tile([1, D], f32)
                for c in range(TC):
                    nc.tensor.matmul(
                        pout[:],
                        w[:, c : c + 1],
                        h_sb[:, c, :],
                        start=(c == 0),
                        stop=(c == TC - 1),
                    )
                # Z = sum over all t of relu'd scores
                pz = pszp.tile([1, 1], f32)
                nc.tensor.matmul(pz[:], zcol[:], ones_col[:], start=True, stop=True)

                zeps = smallp.tile([1, 1], f32, tag="zeps")
                nc.scalar.activation(
                    zeps[:],
                    pz[:],
                    mybir.ActivationFunctionType.Identity,
                    bias=eps_tile[0:1, 0:1],
                )
                zrec = smallp.tile([1, 1], f32, tag="zrec")
                nc.vector.reciprocal(zrec[:], zeps[:])
                res = smallp.tile([1, D], f32, tag="res")
                nc.scalar.mul(res[:], pout[:], zrec[0:1, 0:1])
                nc.sync.dma_start(out_ap[b : b + 1, :], res[:])

    nc.finalize()
    _nc_cache = nc
    return nc


def _run(h_all: np.ndarray, trace: bool = False):
    nc = _build()
    h_all = np.ascontiguousarray(np.asarray(h_all), dtype=np.float32)
    assert h_all.shape == (T, B, D)
    in_maps = [
        {"h": np.ascontiguousarray(h_all[:, c * BPC : (c + 1) * BPC, :])}
        for c in range(NCORES)
    ]
    r = run_bass_kernel_spmd(nc, in_maps, list(range(NCORES)), trace=trace)
    out = np.concatenate([r.results[c]["out"] for c in range(NCORES)], axis=0)
    return out, r


def kernel(h_all: np.ndarray, xin: np.ndarray | None = None) -> np.ndarray:
    out, _ = _run(h_all)
    return out


# revision 34
# speedup vs baseline: 1.1771x; 1.0436x over previous
"""AttentivePooling Trainium2 kernel.

Reference semantics (h_all: [T, B, D] f32, xin unused):
    h_last = h_all[-1]                       # [B, D]
    a[b, t] = <h_all[t, b, :], h_last[b, :]> / sqrt(D)
    r = relu(a)
    w = r / (sum_t r + 1e-9)
    out[b, d] = sum_t w[b, t] * h_all[t, b, d]

Strategy: data-parallel over B across 8 cores (8 batches/core, no
collectives).  Per batch on-device:
  - one 4MB SWDGE DMA loads h_b as 16 SBUF chunks [128(t), 512(d)]
    (t = c*128 + p); a single HWDGE dma_start with 2048 descriptors
    wedges the exec unit, SWDGE handles it
  - PE broadcasts h_last across 128 partitions into PSUM
    (ones[1,128].T @ hl[1,512]); the DVE multiply reads it from PSUM
  - scores: DVE tensor_tensor multiply; the free-dim reduction is
    split between ACT (activation accum_out, scale=1/sqrt(D) folded)
    and DVE (tensor_reduce + tensor_scalar rescale) to balance load.
    (The fused DVE tensor_tensor_reduce crashes the exec unit on HW.)
  - ACT relu with accum_out produces weights + their per-partition sums
  - PE accumulates sum_t w_t * h_t into PSUM [1, 512] (w stationary)
  - PE reduces the weight-sum across partitions via a ones column
  - DVE computes 1/(Z + 1e-9); ACT scales the pooled vector
"""

import numpy as np
from contextlib import ExitStack

import concourse.bass as bass
import concourse.tile as tile
from concourse import bacc, mybir
from concourse.bass_utils import run_bass_kernel_spmd

T, B, D = 2048, 64, 512
NCORES = 8
BPC = B // NCORES  # batches per core
P = 128
TC = T // P  # 16 T-chunks per batch
SCALE = float(1.0 / np.sqrt(np.float32(D)))
DVE_REDUCE_CHUNKS = frozenset({3, 6, 9, 12, 15})  # reduces on DVE; rest on ACT
GP_MULT_CHUNKS = frozenset({2, 5, 8, 11, 14})  # multiplies on GPSIMD; rest on DVE

_nc_cache = None


def _build():
    global _nc_cache
    if _nc_cache is not None:
        return _nc_cache
    nc = bacc.Bacc("TRN2", debug=False, target_bir_lowering=False, num_devices=NCORES)
    h = nc.dram_tensor("h", [T, BPC, D], mybir.dt.float32r, kind="ExternalInput")
    out = nc.dram_tensor("out", [BPC, D], mybir.dt.float32, kind="ExternalOutput")
    h_ap = h.ap()
    out_ap = out.ap()
    f32 = mybir.dt.float32
    f32r = mybir.dt.float32r

    with tile.TileContext(nc) as tc:
        with ExitStack() as ctx:
            hpool = ctx.enter_context(tc.tile_pool(name="h", bufs=4))
            psbcp = ctx.enter_context(tc.tile_pool(name="psb", bufs=2, space="PSUM"))
            tmpp = ctx.enter_context(tc.tile_pool(name="tmp", bufs=6))
            smallp = ctx.enter_context(tc.tile_pool(name="small", bufs=3))
            constp = ctx.enter_context(tc.tile_pool(name="const", bufs=1))
            psoutp = ctx.enter_context(tc.tile_pool(name="pso", bufs=3, space="PSUM"))
            pszp = ctx.enter_context(tc.tile_pool(name="psz", bufs=3, space="PSUM"))

            ones_col = constp.tile([P, 1], f32)
            nc.vector.memset(ones_col[:], 1.0)
            eps_tile = constp.tile([1, 1], f32)
            nc.vector.memset(eps_tile[:], 1e-9)

            def bcast(b):
                # broadcast h_last[b] to all 128 partitions via
                # partition-stride-0 DMA reads straight from DRAM.
                # One copy lands in PSUM (DVE reads it via the PSUM port,
                # halving SBUF read-port pressure), one in SBUF (GPSIMD
                # cannot read PSUM).
                src_bc = h_ap[T - 1 : T, b, :].bitcast(f32).broadcast_to([P, D])
                hlb = smallp.tile([P, D], f32, tag="hlb", name="hlb")
                nc.sync.dma_start(hlb[:], src_bc)
                psb = psbcp.tile([P, D], f32, tag="psb")
                nc.scalar.copy(psb[:], hlb[:])
                return psb, hlb

            HALF = TC // 2

            def load_h(b):
                t = hpool.tile([P, TC, D], f32r, tag="hsb", name="h_sb")
                src_ap = h_ap[:, b, :].rearrange("(c p) d -> p c d", p=P)
                nc.gpsimd.dma_start(t[:, 0:HALF, :], src_ap[:, 0:HALF, :])
                nc.gpsimd.dma_start(t[:, HALF:TC, :], src_ap[:, HALF:TC, :])
                return t

            h_tiles = {}
            for b in range(min(2, BPC)):
                h_tiles[b] = load_h(b)
            psb_tiles = {0: bcast(0)}

            for b in range(BPC):
                h_sb = h_tiles.pop(b)
                psb, hlb = psb_tiles.pop(b)

                # scores: scr[p, c] = sum_d h[t, d] * hl[d] * SCALE
                scr = smallp.tile([P, TC], f32, tag="scr")
                for c in range(TC):
                    tmp = tmpp.tile([P, D], f32, tag="tmp")
                    if c in GP_MULT_CHUNKS:
                        nc.gpsimd.tensor_tensor(
                            tmp[:],
                            h_sb[:, c, :].bitcast(f32),
                            hlb[:],
                            mybir.AluOpType.mult,
                        )
                    else:
                        nc.vector.tensor_tensor(
                            tmp[:],
                            h_sb[:, c, :].bitcast(f32),
                            psb[:],
                            mybir.AluOpType.mult,
                        )
                    if c in DVE_REDUCE_CHUNKS:
                        nc.vector.tensor_reduce(
                            scr[:, c : c + 1],
                            tmp[:],
                            mybir.AxisListType.X,
                            mybir.AluOpType.add,
                        )
                    else:
                        nc.scalar.activation(
                            tmp[:],
                            tmp[:],
                            mybir.ActivationFunctionType.Copy,
                            scale=SCALE,
                            accum_out=scr[:, c : c + 1],
                        )
                # rescale the DVE-reduced columns (ACT ones had SCALE folded)
                for c in sorted(DVE_REDUCE_CHUNKS):
                    nc.vector.tensor_scalar_mul(
                        scr[:, c : c + 1], scr[:, c : c + 1], SCALE
                    )

                # relu + per-partition sum of relu'd scores
                w = smallp.tile([P, TC], f32r, tag="w")
                zcol = smallp.tile([P, 1], f32, tag="z")
                nc.scalar.activation(
                    w[:], scr[:], mybir.ActivationFunctionType.Relu, accum_out=zcol[:]
                )

                # next batch's broadcast goes to PE BEFORE this batch's
                # pooling burst, so the next scores phase is not blocked
                # behind the pooling in PE program order
                if b + 1 < BPC:
                    psb_tiles[b + 1] = bcast(b + 1)
                if b + 2 < BPC:
                    h_tiles[b + 2] = load_h(b + 2)

                # pooled[d] = sum_t w_t * h[t, d] accumulated over chunks
                pout = psoutp.tile([1, D], f32)
                for c in range(TC):
                    nc.tensor.matmul(
                        pout[:],
                        w[:, c : c + 1],
                        h_sb[:, c, :],
                        start=(c == 0),
                        stop=(c == TC - 1),
                    )
                # Z = sum over all t of relu'd scores
                pz = pszp.tile([1, 1], f32)
                nc.tensor.matmul(pz[:], zcol[:], ones_col[:], start=True, stop=True)

                zeps = smallp.tile([1, 1], f32, tag="zeps")
                nc.scalar.activation(
                    zeps[:],
                    pz[:],
                    mybir.ActivationFunctionType.Identity,
                    bias=eps_tile[0:1, 0:1],
                )
                zrec = smallp.tile([1, 1], f32, tag="zrec")
                nc.vector.reciprocal(zrec[:], zeps[:])
                res = smallp.tile([1, D], f32, tag="res")
                nc.scalar.mul(res[:], pout[:], zrec[0:1, 0:1])
                nc.sync.dma_start(out_ap[b : b + 1, :], res[:])

    nc.finalize()
    _nc_cache = nc
    return nc


def _run(h_all: np.ndarray, trace: bool = False):
    nc = _build()
    h_all = np.ascontiguousarray(np.asarray(h_all), dtype=np.float32)
    assert h_all.shape == (T, B, D)
    in_maps = [
        {"h": np.ascontiguousarray(h_all[:, c * BPC : (c + 1) * BPC, :])}
        for c in range(NCORES)
    ]
    r = run_bass_kernel_spmd(nc, in_maps, list(range(NCORES)), trace=trace)
    out = np.concatenate([r.results[c]["out"] for c in range(NCORES)], axis=0)
    return out, r


def kernel(h_all: np.ndarray, xin: np.ndarray | None = None) -> np.ndarray:
    out, _ = _run(h_all)
    return out


# revision 36
# speedup vs baseline: 1.2331x; 1.0475x over previous
"""AttentivePooling Trainium2 kernel.

Reference semantics (h_all: [T, B, D] f32, xin unused):
    h_last = h_all[-1]                       # [B, D]
    a[b, t] = <h_all[t, b, :], h_last[b, :]> / sqrt(D)
    r = relu(a)
    w = r / (sum_t r + 1e-9)
    out[b, d] = sum_t w[b, t] * h_all[t, b, d]

Strategy: data-parallel over B across 8 cores (8 batches/core, no
collectives).  Per batch on-device (pipelined two batches deep):
  - two 2MB SWDGE DMAs load h_b as 16 SBUF chunks [128(t), 512(d)]
    (t = c*128 + p).  (A single HWDGE dma_start with 2048 descriptors
    wedges the exec unit; SWDGE handles it.)
  - h_last[b] is broadcast across the 128 partitions with a
    partition-stride-0 DMA; ACT mirrors it into PSUM so the DVE
    multiplies read it through the PSUM port (halves SBUF read-port
    pressure; fp32 tensor_tensor is otherwise port-bound at 1x).
  - scores: elementwise multiply split DVE (11 chunks) / GPSIMD (5),
    free-dim reduction split ACT activation-accum (11, with the
    1/sqrt(D) scale folded in) / DVE tensor_reduce (5).  (The fused
    DVE tensor_tensor_reduce crashes the exec unit on this HW.)
  - ACT relu with accum_out produces weights + their per-partition sums
  - PE accumulates sum_t w_t * h_t into PSUM [1, 512] with float32r
    matmuls (w stationary): 1 cycle/row vs fp32's 4, at ~1e-4 rounding
  - PE reduces the weight-sum across partitions via a ones column
  - DVE computes 1/(Z + 1e-9); ACT scales the pooled vector
"""

import numpy as np
from contextlib import ExitStack

import concourse.bass as bass
import concourse.tile as tile
from concourse import bacc, mybir
from concourse.bass_utils import run_bass_kernel_spmd

T, B, D = 2048, 64, 512
NCORES = 8
BPC = B // NCORES  # batches per core
P = 128
TC = T // P  # 16 T-chunks per batch
SCALE = float(1.0 / np.sqrt(np.float32(D)))
DVE_REDUCE_CHUNKS = frozenset({3, 6, 9, 12, 15})  # reduces on DVE; rest on ACT
GP_MULT_CHUNKS = frozenset({2, 4, 7, 9, 12, 14})  # multiplies on GPSIMD; rest on DVE

_nc_cache = None


def _build():
    global _nc_cache
    if _nc_cache is not None:
        return _nc_cache
    nc = bacc.Bacc("TRN2", debug=False, target_bir_lowering=False, num_devices=NCORES)
    h = nc.dram_tensor("h", [T, BPC, D], mybir.dt.float32r, kind="ExternalInput")
    out = nc.dram_tensor("out", [BPC, D], mybir.dt.float32, kind="ExternalOutput")
    h_ap = h.ap()
    out_ap = out.ap()
    f32 = mybir.dt.float32
    f32r = mybir.dt.float32r

    with tile.TileContext(nc) as tc:
        with ExitStack() as ctx:
            hpool = ctx.enter_context(tc.tile_pool(name="h", bufs=5))
            psbcp = ctx.enter_context(tc.tile_pool(name="psb", bufs=2, space="PSUM"))
            tmpp = ctx.enter_context(tc.tile_pool(name="tmp", bufs=6))
            smallp = ctx.enter_context(tc.tile_pool(name="small", bufs=3))
            constp = ctx.enter_context(tc.tile_pool(name="const", bufs=1))
            psoutp = ctx.enter_context(tc.tile_pool(name="pso", bufs=3, space="PSUM"))
            pszp = ctx.enter_context(tc.tile_pool(name="psz", bufs=3, space="PSUM"))

            ones_col = constp.tile([P, 1], f32)
            nc.vector.memset(ones_col[:], 1.0)
            eps_tile = constp.tile([1, 1], f32)
            nc.vector.memset(eps_tile[:], 1e-9)

            def bcast(b):
                # broadcast h_last[b] to all 128 partitions via
                # partition-stride-0 DMA reads straight from DRAM.
                # One copy lands in PSUM (DVE reads it via the PSUM port,
                # halving SBUF read-port pressure), one in SBUF (GPSIMD
                # cannot read PSUM).
                src_bc = h_ap[T - 1 : T, b, :].bitcast(f32).broadcast_to([P, D])
                hlb = smallp.tile([P, D], f32, tag="hlb", name="hlb")
                nc.sync.dma_start(hlb[:], src_bc)
                psb = psbcp.tile([P, D], f32, tag="psb")
                nc.scalar.copy(psb[:], hlb[:])
                return psb, hlb

            HALF = TC // 2

            def load_h(b):
                t = hpool.tile([P, TC, D], f32r, tag="hsb", name="h_sb")
                src_ap = h_ap[:, b, :].rearrange("(c p) d -> p c d", p=P)
                nc.gpsimd.dma_start(t[:, 0:HALF, :], src_ap[:, 0:HALF, :])
                nc.gpsimd.dma_start(t[:, HALF:TC, :], src_ap[:, HALF:TC, :])
                return t

            h_tiles = {}
            for b in range(min(2, BPC)):
                h_tiles[b] = load_h(b)
            psb_tiles = {0: bcast(0)}

            for b in range(BPC):
                h_sb = h_tiles.pop(b)
                psb, hlb = psb_tiles.pop(b)

                # scores: scr[p, c] = sum_d h[t, d] * hl[d] * SCALE
                scr = smallp.tile([P, TC], f32, tag="scr")
                for c in range(TC):
                    tmp = tmpp.tile([P, D], f32, tag="tmp")
                    if c in GP_MULT_CHUNKS:
                        nc.gpsimd.tensor_tensor(
                            tmp[:],
                            h_sb[:, c, :].bitcast(f32),
                            hlb[:],
                            mybir.AluOpType.mult,
                        )
                    else:
                        nc.vector.tensor_tensor(
                            tmp[:],
                            h_sb[:, c, :].bitcast(f32),
                            psb[:],
                            mybir.AluOpType.mult,
                        )
                    if c in DVE_REDUCE_CHUNKS:
                        nc.vector.tensor_reduce(
                            scr[:, c : c + 1],
                            tmp[:],
                            mybir.AxisListType.X,
                            mybir.AluOpType.add,
                        )
                    else:
                        nc.scalar.activation(
                            tmp[:],
                            tmp[:],
                            mybir.ActivationFunctionType.Copy,
                            scale=SCALE,
                            accum_out=scr[:, c : c + 1],
                        )
                # rescale the DVE-reduced columns (ACT ones had SCALE folded)
                for c in sorted(DVE_REDUCE_CHUNKS):
                    nc.vector.tensor_scalar_mul(
                        scr[:, c : c + 1], scr[:, c : c + 1], SCALE
                    )

                # relu + per-partition sum of relu'd scores
                w = smallp.tile([P, TC], f32r, tag="w")
                zcol = smallp.tile([P, 1], f32, tag="z")
                nc.scalar.activation(
                    w[:], scr[:], mybir.ActivationFunctionType.Relu, accum_out=zcol[:]
                )

                # next batch's broadcast goes to PE BEFORE this batch's
                # pooling burst, so the next scores phase is not blocked
                # behind the pooling in PE program order
                if b + 1 < BPC:
                    psb_tiles[b + 1] = bcast(b + 1)
                if b + 2 < BPC:
                    h_tiles[b + 2] = load_h(b + 2)

                # pooled[d] = sum_t w_t * h[t, d] accumulated over chunks
                pout = psoutp.tile([1, D], f32)
                for c in range(TC):
                    nc.tensor.matmul(
                        pout[:],
                        w[:, c : c + 1],
                        h_sb[:, c, :],
                        start=(c == 0),
                        stop=(c == TC - 1),
                    )
                # Z = sum over all t of relu'd scores
                pz = pszp.tile([1, 1], f32)
                nc.tensor.matmul(pz[:], zcol[:], ones_col[:], start=True, stop=True)

                zeps = smallp.tile([1, 1], f32, tag="zeps")
                nc.scalar.activation(
                    zeps[:],
                    pz[:],
                    mybir.ActivationFunctionType.Identity,
                    bias=eps_tile[0:1, 0:1],
                )
                zrec = smallp.tile([1, 1], f32, tag="zrec")
                nc.vector.reciprocal(zrec[:], zeps[:])
                res = smallp.tile([1, D], f32, tag="res")
                nc.scalar.mul(res[:], pout[:], zrec[0:1, 0:1])
                nc.sync.dma_start(out_ap[b : b + 1, :], res[:])

    nc.finalize()
    _nc_cache = nc
    return nc


def _run(h_all: np.ndarray, trace: bool = False):
    nc = _build()
    h_all = np.ascontiguousarray(np.asarray(h_all), dtype=np.float32)
    assert h_all.shape == (T, B, D)
    in_maps = [
        {"h": np.ascontiguousarray(h_all[:, c * BPC : (c + 1) * BPC, :])}
        for c in range(NCORES)
    ]
    r = run_bass_kernel_spmd(nc, in_maps, list(range(NCORES)), trace=trace)
    out = np.concatenate([r.results[c]["out"] for c in range(NCORES)], axis=0)
    return out, r


def kernel(h_all: np.ndarray, xin: np.ndarray | None = None) -> np.ndarray:
    out, _ = _run(h_all)
    return out
